# revision 5
# baseline (speedup 1.0000x reference)
"""Self-contained Trainium2 Bass kernel for deformable conv 2d.

kernel(x, offset, weight) -> out, matching the jax reference:
  x[2,256,64,64] f32, offset[2,18,64,64] f32, weight[256,256,3,3] f32
  -> out[2,256,64,64] f32 (KH=KW=3, stride=1, pad=1, dil=1, DG=1).

Runs SPMD on 8 NeuronCores, data-parallel: core = (batch, spatial quarter).

Device pipeline (per core, taps outer, 8 position-groups of 128 inner):
  - per tap k: DVE builds diag(w_corner) tiles for all 8 groups from
    host-computed bilinear weights (one broadcast-AP tensor_tensor).
  - 8 indirect gathers (one per group) fetch each sample's 2x2 bilinear
    window (4 corners x 256ch bf16) into [128 pos, 4*256].
  - PE does blend+transpose in one step: psum[ch,pos] += gt_a^T @ diag(w_a)
    accumulated over the 4 corners (regular matmul with diagonal rhs
    scales each transposed column by its sample weight).
  - Act copies blended psum -> SBUF bf16 rhsT; PE contracts the tap's
    (cin-chunk) pieces with the conv weights into the persistent psum out
    at 512-wide free dim.
Host precomputes gather indices and corner weights from the offsets.
"""

import sys

for _p in ("/opt/trn_rl_repo",):
    if _p not in sys.path:
        sys.path.insert(0, _p)


import numpy as np
import ml_dtypes

import concourse.bass as bass
import concourse.mybir as mybir
import concourse.tile as tile

F32 = mybir.dt.float32
BF16 = mybir.dt.bfloat16
I32 = mybir.dt.int32

N, CIN, H, W = 2, 256, 64, 64
COUT = 256
KH = KW = 3
K = KH * KW
S = H * W            # 4096 output positions per batch
SLOC = S // 4        # 1024 per core
NG = 8               # position groups per core (128 positions each)
NT = K * NG          # 72 (tap, group) gather slots

AluOp = mybir.AluOpType


def build_core_kernel(nc, tc, outs, ins):
    """Emit the per-core kernel. ins/outs are dicts of DRAM APs."""
    from contextlib import ExitStack

    xi = ins["xi"]          # [4096, 512] bf16 y-pair-interleaved image
    wT = ins["wT"]          # [2304, 256] bf16 lhsT
    ridx_d = ins["ridx"]    # [128, 72] i32 gather rows, col = k*8+g
    cw_d = ins["cw"]        # [128, 9, 32] bf16 corner weights (k; g*4+a)
    ident_d = ins["ident"]  # [128, 128] bf16 identity
    out = outs["out"]       # [128, 2, 1024] f32  (pos index = g*128+p)

    ctx = ExitStack()
    sp = ctx.enter_context(tc.tile_pool(name="static", bufs=1))
    gp = ctx.enter_context(tc.tile_pool(name="gather", bufs=16))
    dgp = ctx.enter_context(tc.tile_pool(name="diag", bufs=4))
    rp = ctx.enter_context(tc.tile_pool(name="rhsT", bufs=2))
    bp = ctx.enter_context(tc.tile_pool(name="bpsum", bufs=2, space="PSUM"))
    cp = ctx.enter_context(tc.tile_pool(name="cpsum", bufs=1, space="PSUM"))
    op = ctx.enter_context(tc.tile_pool(name="osb", bufs=1))

    # ---- static loads (small control inputs first) ----
    ridx = sp.tile([128, NT], I32, name="ridx")
    nc.sync.dma_start(ridx[:], ridx_d)
    cw = sp.tile([128, K, 4 * NG], BF16, name="cw")
    nc.sync.dma_start(cw[:], cw_d)
    ident = sp.tile([128, 128], BF16, name="ident")
    nc.sync.dma_start(ident[:], ident_d)
    wT_s = sp.tile([128, 18, 256], BF16, name="wT_s")
    nc.sync.dma_start(wT_s[:], wT.rearrange("(j p) o -> p j o", p=128))

    id_bc = ident[:].unsqueeze(1).broadcast_to([128, 4 * NG, 128])

    po = cp.tile([128, 2, SLOC], F32, name="po", space="PSUM")

    for k in range(K):
        # diag tiles for this tap: diag[p, (g,a), f] = I[p,f]*cw[p,k,(g,a)]
        diag = dgp.tile([128, 4 * NG, 128], BF16, name="diag")
        cw_bc = cw[:, k].unsqueeze(-1).broadcast_to([128, 4 * NG, 128])
        nc.vector.tensor_tensor(diag[:], id_bc, cw_bc, AluOp.mult)

        rhsT = rp.tile([128, 2, NG, 128], BF16, name="rhsT")
        for gh in range(2):            # groups in halves of 4
            pb = bp.tile([128, 2, 4, 128], F32, name="pb", space="PSUM")
            for gg in range(4):
                g = gh * 4 + gg
                gt = gp.tile([128, 4, 256], BF16, name="gt")
                t = k * NG + g
                nc.gpsimd.indirect_dma_start(
                    out=gt[:].rearrange("p a b -> p (a b)"),
                    out_offset=None,
                    in_=xi,
                    in_offset=bass.IndirectOffsetOnAxis(
                        ap=ridx[:, t : t + 1], axis=0
                    ),
                )
                # blend + transpose: psum[ch,pos] += gt_a^T @ diag(w_a)
                for cc in range(2):
                    for a in range(4):
                        nc.tensor.matmul(
                            pb[:, cc, gg, :],
                            gt[:, a, cc * 128 : (cc + 1) * 128],
                            diag[:, 4 * g + a, :],
                            start=(a == 0),
                            stop=(a == 3),
                        )
            nc.scalar.copy(rhsT[:, :, gh * 4 : gh * 4 + 4, :], pb[:])
        # conv: accumulate this tap's 2 cin-chunks into the persistent psum
        for h in range(2):
            for cc in range(2):
                j = 2 * k + cc
                for sh in range(2):
                    nc.tensor.matmul(
                        po[:, h, sh * 512 : (sh + 1) * 512],
                        wT_s[:, j, h * 128 : (h + 1) * 128],
                        rhsT[:, cc].rearrange("p g b -> p (g b)")[
                            :, sh * 512 : (sh + 1) * 512
                        ],
                        start=(j == 0),
                        stop=(j == 17),
                    )
    for h in range(2):
        osb = op.tile([128, SLOC], F32, name=f"osb{h}")
        nc.scalar.copy(osb[:], po[:, h, :])
        nc.sync.dma_start(out[:, h, :], osb[:])

    ctx.close()


# ---------------- host-side prep ----------------

def core_inputs(x, offset, weight):
    """Full inputs (np f32) -> list of 8 per-core input dicts."""
    bf = ml_dtypes.bfloat16
    x = np.asarray(x, np.float32)
    offset = np.asarray(offset, np.float32)
    weight = np.asarray(weight, np.float32)

    # y-pair-interleaved channels-last images, bf16: xi[r] = [x[r], x[r+64]]
    xis = []
    for n in range(N):
        xcl = np.ascontiguousarray(x[n].reshape(CIN, S).T)  # [4096, 256]
        xi = np.zeros((S, 2 * CIN), np.float32)
        xi[:, :CIN] = xcl
        xi[: S - W, CIN:] = xcl[W:]
        xis.append(xi.astype(bf))

    # lhsT [k*256+c, o]
    wk = weight.reshape(COUT, CIN, K)           # [o, c, k]
    wT = np.ascontiguousarray(wk.transpose(2, 1, 0).reshape(K * CIN, COUT)).astype(bf)

    ident = np.eye(128, dtype=bf)

    # sample coordinates: py/px [K, S] per batch
    off = offset.reshape(N, K, 2, S)
    ky, kx = np.meshgrid(np.arange(KH), np.arange(KW), indexing="ij")
    ky = ky.reshape(K, 1).astype(np.float32)
    kx = kx.reshape(K, 1).astype(np.float32)
    ho, wo = np.meshgrid(np.arange(H), np.arange(W), indexing="ij")
    base_y = ho.reshape(1, S).astype(np.float32) - 1.0 + ky   # [K, S]
    base_x = wo.reshape(1, S).astype(np.float32) - 1.0 + kx

    ins = []
    for core in range(8):
        n, qtr = core // 4, core % 4
        sl = slice(qtr * SLOC, (qtr + 1) * SLOC)
        py = base_y[:, sl] + off[n, :, 0, sl]   # [K, 1024]
        px = base_x[:, sl] + off[n, :, 1, sl]

        fy = np.floor(py)
        fx = np.floor(px)
        ly, lx = py - fy, px - fx
        hy, hx = 1.0 - ly, 1.0 - lx
        wy_c = np.clip(fy, 0.0, 62.0)           # window start rows
        wx_c = np.clip(fx, 0.0, 62.0)

        def sw(f, l, h, wc):
            """weights of window slots 0/1 along one axis, validity folded."""
            v0 = (f >= 0) & (f <= 63)
            v1 = (f + 1 >= 0) & (f + 1 <= 63)
            w0 = h * v0                          # corner f
            w1 = l * v1                          # corner f+1
            ws = []
            for s_ in (0, 1):
                c = wc + s_
                ws.append(w0 * (c == f) + w1 * (c == f + 1))
            return ws                            # [2][K, 1024]

        wys = sw(fy, ly, hy, wy_c)
        wxs = sw(fx, lx, hx, wx_c)
        rows = (wy_c * 64.0 + wx_c).astype(np.int32)        # [K, 1024]

        # device layouts: position p*8+g <-> (partition p, group g)
        def lay(a):                               # [K, 1024] -> [128, K, 8]
            return np.ascontiguousarray(
                a.reshape(K, 128, NG).transpose(1, 0, 2)
            )

        ridx = lay(rows).reshape(128, K * NG).astype(np.int32)  # col k*8+g
        cwa = np.stack(
            [wys[0] * wxs[0], wys[1] * wxs[0], wys[0] * wxs[1], wys[1] * wxs[1]],
            axis=0,
        )                                          # [4, K, 1024]
        # -> [128, K, 8*4+a]  (column within tap = g*4 + a)
        cwk = np.ascontiguousarray(
            cwa.reshape(4, K, 128, NG).transpose(2, 1, 3, 0).reshape(128, K, 4 * NG)
        ).astype(bf)

        ins.append({
            "xi": xis[n],
            "wT": wT,
            "ident": ident,
            "ridx": ridx,
            "cw": cwk,
        })
    return ins


def assemble(results):
    """list of 8 per-core {'out': [128,2,1024] f32} -> [2,256,64,64] f32."""
    out = np.zeros((N, COUT, S), np.float32)
    for core in range(8):
        n, qtr = core // 4, core % 4
        o = np.asarray(results[core]["out"])          # [oc, h, g*128+p]
        o = o.transpose(1, 0, 2).reshape(COUT, SLOC)  # [cout, g*128+p]
        # position within quarter = p*8+g
        o = o.reshape(COUT, NG, 128).transpose(0, 2, 1).reshape(COUT, SLOC)
        out[n, :, qtr * SLOC : (qtr + 1) * SLOC] = o
    return out.reshape(N, COUT, H, W)


def declare_io(nc):
    ins = {
        "xi": nc.dram_tensor("xi", [S, 2 * CIN], BF16, kind="ExternalInput").ap(),
        "wT": nc.dram_tensor("wT", [K * CIN, COUT], BF16, kind="ExternalInput").ap(),
        "ridx": nc.dram_tensor("ridx", [128, NT], I32, kind="ExternalInput").ap(),
        "cw": nc.dram_tensor("cw", [128, K, 4 * NG], BF16, kind="ExternalInput").ap(),
        "ident": nc.dram_tensor("ident", [128, 128], BF16, kind="ExternalInput").ap(),
    }
    outs = {
        "out": nc.dram_tensor("out", [128, 2, SLOC], F32, kind="ExternalOutput").ap(),
    }
    return outs, ins


def build_module():
    from concourse import bacc

    nc = bacc.Bacc(
        "TRN2",
        target_bir_lowering=False,
        debug=False,
        num_devices=8,
        dynamic_dma_scratch_size=65536,
    )
    outs, ins = declare_io(nc)
    with tile.TileContext(nc) as tc:
        build_core_kernel(nc, tc, outs, ins)
    nc.compile()
    return nc


_NC_CACHE = []


def kernel(x, offset, weight):
    """Full (unsharded) inputs -> full output, computed on 8 NeuronCores."""
    import time

    from concourse.bass_utils import run_bass_kernel_spmd

    if not _NC_CACHE:
        _NC_CACHE.append(build_module())
    nc = _NC_CACHE[0]
    core_ins = core_inputs(x, offset, weight)
    last = None
    for attempt in range(3):
        try:
            res = run_bass_kernel_spmd(nc, core_ins, core_ids=list(range(8)))
            return assemble(res.results)
        except Exception as e:  # transient device-session failures
            last = e
            time.sleep(2.0 * (attempt + 1))
    raise last


# revision 6
# speedup vs baseline: 1.0060x; 1.0060x over previous
"""Self-contained Trainium2 Bass kernel for deformable conv 2d.

kernel(x, offset, weight) -> out, matching the jax reference:
  x[2,256,64,64] f32, offset[2,18,64,64] f32, weight[256,256,3,3] f32
  -> out[2,256,64,64] f32 (KH=KW=3, stride=1, pad=1, dil=1, DG=1).

Runs SPMD on 8 NeuronCores, data-parallel: core = (batch, spatial quarter).

Device pipeline (per core, taps outer, 8 position-groups of 128 inner):
  - per tap k: DVE builds diag(w_corner) tiles for all 8 groups from
    host-computed bilinear weights (one broadcast-AP tensor_tensor).
  - 8 indirect gathers (one per group) fetch each sample's 2x2 bilinear
    window (4 corners x 256ch bf16) into [128 pos, 4*256].
  - PE does blend+transpose in one step: psum[ch,pos] += gt_a^T @ diag(w_a)
    accumulated over the 4 corners (regular matmul with diagonal rhs
    scales each transposed column by its sample weight).
  - Act copies blended psum -> SBUF bf16 rhsT; PE contracts the tap's
    (cin-chunk) pieces with the conv weights into the persistent psum out
    at 512-wide free dim.
Host precomputes gather indices and corner weights from the offsets.
"""

import sys

for _p in ("/opt/trn_rl_repo",):
    if _p not in sys.path:
        sys.path.insert(0, _p)


import numpy as np
import ml_dtypes

import concourse.bass as bass
import concourse.mybir as mybir
import concourse.tile as tile

F32 = mybir.dt.float32
BF16 = mybir.dt.bfloat16
I32 = mybir.dt.int32

N, CIN, H, W = 2, 256, 64, 64
COUT = 256
KH = KW = 3
K = KH * KW
S = H * W            # 4096 output positions per batch
SLOC = S // 4        # 1024 per core
NG = 8               # position groups per core (128 positions each)
NT = K * NG          # 72 (tap, group) gather slots

AluOp = mybir.AluOpType


def build_core_kernel(nc, tc, outs, ins):
    """Emit the per-core kernel. ins/outs are dicts of DRAM APs."""
    from contextlib import ExitStack

    xi = ins["xi"]          # [4096, 512] bf16 y-pair-interleaved image
    wT = ins["wT"]          # [2304, 256] bf16 lhsT
    ridx_d = ins["ridx"]    # [128, 72] i32 gather rows, col = k*8+g
    cw_d = ins["cw"]        # [128, 9, 32] bf16 corner weights (k; g*4+a)
    ident_d = ins["ident"]  # [128, 128] bf16 identity
    out = outs["out"]       # [128, 2, 1024] f32  (pos index = g*128+p)

    ctx = ExitStack()
    sp = ctx.enter_context(tc.tile_pool(name="static", bufs=1))
    gp = ctx.enter_context(tc.tile_pool(name="gather", bufs=12))
    dgp = ctx.enter_context(tc.tile_pool(name="diag", bufs=4))
    rp = ctx.enter_context(tc.tile_pool(name="rhsT", bufs=2))
    bp = ctx.enter_context(tc.tile_pool(name="bpsum", bufs=2, space="PSUM"))
    cp = ctx.enter_context(tc.tile_pool(name="cpsum", bufs=1, space="PSUM"))
    op = ctx.enter_context(tc.tile_pool(name="osb", bufs=1))

    # ---- static loads (small control inputs first) ----
    ridx = sp.tile([128, NT], I32, name="ridx")
    nc.sync.dma_start(ridx[:], ridx_d)
    cw = sp.tile([128, K, 4 * NG], BF16, name="cw")
    nc.sync.dma_start(cw[:], cw_d)
    ident = sp.tile([128, 128], BF16, name="ident")
    nc.sync.dma_start(ident[:], ident_d)
    wT_s = sp.tile([128, 18, 256], BF16, name="wT_s")
    nc.sync.dma_start(wT_s[:], wT.rearrange("(j p) o -> p j o", p=128))

    id_bc = ident[:].unsqueeze(1).broadcast_to([128, 4 * NG, 128])

    po = cp.tile([128, 2, SLOC], F32, name="po", space="PSUM")

    for k in range(K):
        # diag tiles for this tap: diag[p, (g,a), f] = I[p,f]*cw[p,k,(g,a)]
        diag = dgp.tile([128, 4 * NG, 128], BF16, name="diag")
        cw_bc = cw[:, k].unsqueeze(-1).broadcast_to([128, 4 * NG, 128])
        nc.vector.tensor_tensor(diag[:], id_bc, cw_bc, AluOp.mult)

        rhsT = rp.tile([128, 2, NG, 128], BF16, name="rhsT")
        for gh in range(2):            # groups in halves of 4
            pb = bp.tile([128, 2, 4, 128], F32, name="pb", space="PSUM")
            for gg in range(4):
                g = gh * 4 + gg
                gt = gp.tile([128, 4, 256], BF16, name="gt")
                t = k * NG + g
                nc.gpsimd.indirect_dma_start(
                    out=gt[:].rearrange("p a b -> p (a b)"),
                    out_offset=None,
                    in_=xi,
                    in_offset=bass.IndirectOffsetOnAxis(
                        ap=ridx[:, t : t + 1], axis=0
                    ),
                )
                # blend + transpose: psum[ch,pos] += gt_a^T @ diag(w_a)
                for cc in range(2):
                    for a in range(4):
                        nc.tensor.matmul(
                            pb[:, cc, gg, :],
                            gt[:, a, cc * 128 : (cc + 1) * 128],
                            diag[:, 4 * g + a, :],
                            start=(a == 0),
                            stop=(a == 3),
                        )
            nc.scalar.copy(rhsT[:, :, gh * 4 : gh * 4 + 4, :], pb[:])
        # conv: accumulate this tap's 2 cin-chunks into the persistent psum
        for h in range(2):
            for cc in range(2):
                j = 2 * k + cc
                for sh in range(2):
                    nc.tensor.matmul(
                        po[:, h, sh * 512 : (sh + 1) * 512],
                        wT_s[:, j, h * 128 : (h + 1) * 128],
                        rhsT[:, cc].rearrange("p g b -> p (g b)")[
                            :, sh * 512 : (sh + 1) * 512
                        ],
                        start=(j == 0),
                        stop=(j == 17),
                    )
    for h in range(2):
        osb = op.tile([128, SLOC], F32, name=f"osb{h}")
        nc.scalar.copy(osb[:], po[:, h, :])
        nc.sync.dma_start(out[:, h, :], osb[:])

    ctx.close()


# ---------------- host-side prep ----------------

def core_inputs(x, offset, weight):
    """Full inputs (np f32) -> list of 8 per-core input dicts."""
    bf = ml_dtypes.bfloat16
    x = np.asarray(x, np.float32)
    offset = np.asarray(offset, np.float32)
    weight = np.asarray(weight, np.float32)

    # y-pair-interleaved channels-last images, bf16: xi[r] = [x[r], x[r+64]]
    xis = []
    for n in range(N):
        xcl = np.ascontiguousarray(x[n].reshape(CIN, S).T)  # [4096, 256]
        xi = np.zeros((S, 2 * CIN), np.float32)
        xi[:, :CIN] = xcl
        xi[: S - W, CIN:] = xcl[W:]
        xis.append(xi.astype(bf))

    # lhsT [k*256+c, o]
    wk = weight.reshape(COUT, CIN, K)           # [o, c, k]
    wT = np.ascontiguousarray(wk.transpose(2, 1, 0).reshape(K * CIN, COUT)).astype(bf)

    ident = np.eye(128, dtype=bf)

    # sample coordinates: py/px [K, S] per batch
    off = offset.reshape(N, K, 2, S)
    ky, kx = np.meshgrid(np.arange(KH), np.arange(KW), indexing="ij")
    ky = ky.reshape(K, 1).astype(np.float32)
    kx = kx.reshape(K, 1).astype(np.float32)
    ho, wo = np.meshgrid(np.arange(H), np.arange(W), indexing="ij")
    base_y = ho.reshape(1, S).astype(np.float32) - 1.0 + ky   # [K, S]
    base_x = wo.reshape(1, S).astype(np.float32) - 1.0 + kx

    ins = []
    for core in range(8):
        n, qtr = core // 4, core % 4
        sl = slice(qtr * SLOC, (qtr + 1) * SLOC)
        py = base_y[:, sl] + off[n, :, 0, sl]   # [K, 1024]
        px = base_x[:, sl] + off[n, :, 1, sl]

        fy = np.floor(py)
        fx = np.floor(px)
        ly, lx = py - fy, px - fx
        hy, hx = 1.0 - ly, 1.0 - lx
        wy_c = np.clip(fy, 0.0, 62.0)           # window start rows
        wx_c = np.clip(fx, 0.0, 62.0)

        def sw(f, l, h, wc):
            """weights of window slots 0/1 along one axis, validity folded."""
            v0 = (f >= 0) & (f <= 63)
            v1 = (f + 1 >= 0) & (f + 1 <= 63)
            w0 = h * v0                          # corner f
            w1 = l * v1                          # corner f+1
            ws = []
            for s_ in (0, 1):
                c = wc + s_
                ws.append(w0 * (c == f) + w1 * (c == f + 1))
            return ws                            # [2][K, 1024]

        wys = sw(fy, ly, hy, wy_c)
        wxs = sw(fx, lx, hx, wx_c)
        rows = (wy_c * 64.0 + wx_c).astype(np.int32)        # [K, 1024]

        # device layouts: position p*8+g <-> (partition p, group g)
        def lay(a):                               # [K, 1024] -> [128, K, 8]
            return np.ascontiguousarray(
                a.reshape(K, 128, NG).transpose(1, 0, 2)
            )

        ridx = lay(rows).reshape(128, K * NG).astype(np.int32)  # col k*8+g
        cwa = np.stack(
            [wys[0] * wxs[0], wys[1] * wxs[0], wys[0] * wxs[1], wys[1] * wxs[1]],
            axis=0,
        )                                          # [4, K, 1024]
        # -> [128, K, 8*4+a]  (column within tap = g*4 + a)
        cwk = np.ascontiguousarray(
            cwa.reshape(4, K, 128, NG).transpose(2, 1, 3, 0).reshape(128, K, 4 * NG)
        ).astype(bf)

        ins.append({
            "xi": xis[n],
            "wT": wT,
            "ident": ident,
            "ridx": ridx,
            "cw": cwk,
        })
    return ins


def assemble(results):
    """list of 8 per-core {'out': [128,2,1024] f32} -> [2,256,64,64] f32."""
    out = np.zeros((N, COUT, S), np.float32)
    for core in range(8):
        n, qtr = core // 4, core % 4
        o = np.asarray(results[core]["out"])          # [oc, h, g*128+p]
        o = o.transpose(1, 0, 2).reshape(COUT, SLOC)  # [cout, g*128+p]
        # position within quarter = p*8+g
        o = o.reshape(COUT, NG, 128).transpose(0, 2, 1).reshape(COUT, SLOC)
        out[n, :, qtr * SLOC : (qtr + 1) * SLOC] = o
    return out.reshape(N, COUT, H, W)


def declare_io(nc):
    ins = {
        "xi": nc.dram_tensor("xi", [S, 2 * CIN], BF16, kind="ExternalInput").ap(),
        "wT": nc.dram_tensor("wT", [K * CIN, COUT], BF16, kind="ExternalInput").ap(),
        "ridx": nc.dram_tensor("ridx", [128, NT], I32, kind="ExternalInput").ap(),
        "cw": nc.dram_tensor("cw", [128, K, 4 * NG], BF16, kind="ExternalInput").ap(),
        "ident": nc.dram_tensor("ident", [128, 128], BF16, kind="ExternalInput").ap(),
    }
    outs = {
        "out": nc.dram_tensor("out", [128, 2, SLOC], F32, kind="ExternalOutput").ap(),
    }
    return outs, ins


def build_module():
    from concourse import bacc

    nc = bacc.Bacc(
        "TRN2",
        target_bir_lowering=False,
        debug=False,
        num_devices=8,
        dynamic_dma_scratch_size=65536,
    )
    outs, ins = declare_io(nc)
    with tile.TileContext(nc) as tc:
        build_core_kernel(nc, tc, outs, ins)
    nc.compile()
    return nc


_NC_CACHE = []


def kernel(x, offset, weight):
    """Full (unsharded) inputs -> full output, computed on 8 NeuronCores."""
    import time

    from concourse.bass_utils import run_bass_kernel_spmd

    if not _NC_CACHE:
        _NC_CACHE.append(build_module())
    nc = _NC_CACHE[0]
    core_ins = core_inputs(x, offset, weight)
    last = None
    for attempt in range(3):
        try:
            res = run_bass_kernel_spmd(nc, core_ins, core_ids=list(range(8)))
            return assemble(res.results)
        except Exception as e:  # transient device-session failures
            last = e
            time.sleep(2.0 * (attempt + 1))
    raise last


# revision 12
# speedup vs baseline: 1.2315x; 1.2241x over previous
"""Self-contained Trainium2 Bass kernel for deformable conv 2d.

kernel(x, offset, weight) -> out, matching the jax reference:
  x[2,256,64,64] f32, offset[2,18,64,64] f32, weight[256,256,3,3] f32
  -> out[2,256,64,64] f32 (KH=KW=3, stride=1, pad=1, dil=1, DG=1).

Runs SPMD on 8 NeuronCores, data-parallel: core = (batch, spatial quarter).

Device pipeline (per core, taps outer, 8 position-groups of 128 inner):
  - per tap k: DVE builds diag(w_corner) tiles for all 8 groups from
    host-computed bilinear weights (one broadcast-AP tensor_tensor).
  - 8 indirect gathers (one per group) fetch each sample's 2x2 bilinear
    window (4 corners x 256ch bf16) into [128 pos, 4*256].
  - PE does blend+transpose in one step: psum[ch,pos] += gt_a^T @ diag(w_a)
    accumulated over the 4 corners (regular matmul with diagonal rhs
    scales each transposed column by its sample weight).
  - Act copies blended psum -> SBUF bf16 rhsT; PE contracts the tap's
    (cin-chunk) pieces with the conv weights into the persistent psum out
    at 512-wide free dim.
Host precomputes gather indices and corner weights from the offsets.
"""

import sys

for _p in ("/opt/trn_rl_repo",):
    if _p not in sys.path:
        sys.path.insert(0, _p)


import numpy as np
import ml_dtypes

import concourse.bass as bass
import concourse.mybir as mybir
import concourse.tile as tile

F32 = mybir.dt.float32
BF16 = mybir.dt.bfloat16
I32 = mybir.dt.int32

N, CIN, H, W = 2, 256, 64, 64
COUT = 256
KH = KW = 3
K = KH * KW
S = H * W            # 4096 output positions per batch
SLOC = S // 4        # 1024 per core
NG = 8               # position groups per core (128 positions each)
NT = K * NG          # 72 (tap, group) gather slots

AluOp = mybir.AluOpType


def build_core_kernel(nc, tc, outs, ins):
    """Emit the per-core kernel. ins/outs are dicts of DRAM APs."""
    from contextlib import ExitStack

    xi = ins["xi"]          # [4096, 512] bf16 y-pair-interleaved image
    wT = ins["wT"]          # [2304, 256] bf16 lhsT
    ridx_d = ins["ridx"]    # [128, 72] i32 gather rows, col = k*8+g
    cw_d = ins["cw"]        # [128, 9, 32] bf16 corner weights (k; g*4+a)
    ident_d = ins["ident"]  # [128, 128] bf16 identity
    out = outs["out"]       # [128, 2, 1024] f32  (pos index = g*128+p)

    ctx = ExitStack()
    sp = ctx.enter_context(tc.tile_pool(name="static", bufs=1))
    gp = ctx.enter_context(tc.tile_pool(name="gather", bufs=12))
    dgp = ctx.enter_context(tc.tile_pool(name="diag", bufs=4))  # half-tap tiles
    rp = ctx.enter_context(tc.tile_pool(name="rhsT", bufs=2))
    bp = ctx.enter_context(tc.tile_pool(name="bpsum", bufs=2, space="PSUM"))
    cp = ctx.enter_context(tc.tile_pool(name="cpsum", bufs=1, space="PSUM"))
    op = ctx.enter_context(tc.tile_pool(name="osb", bufs=1))

    # ---- static loads (small control inputs first) ----
    ridx = sp.tile([128, NT], I32, name="ridx")
    nc.sync.dma_start(ridx[:], ridx_d)
    cw = sp.tile([128, K, 4 * NG], BF16, name="cw")
    nc.sync.dma_start(cw[:], cw_d)
    ident = sp.tile([128, 128], BF16, name="ident")
    nc.sync.dma_start(ident[:], ident_d)
    wT_s = sp.tile([128, 18, 256], BF16, name="wT_s")
    nc.sync.dma_start(wT_s[:], wT.rearrange("(j p) o -> p j o", p=128))

    id_bc = ident[:].unsqueeze(1).broadcast_to([128, 16, 128])

    po = cp.tile([128, 2, SLOC], F32, name="po", space="PSUM")

    for k in range(K):
        rhsT = rp.tile([128, 2, NG, 128], BF16, name="rhsT")
        for gh in range(2):            # groups in halves of 4
            # diag tiles for this half: diag[p, (g,a), f] = I[p,f]*cw[p,k,(g,a)]
            diag = dgp.tile([128, 16, 128], BF16, name="diag")
            cw_bc = (
                cw[:, k, gh * 16 : gh * 16 + 16]
                .unsqueeze(-1)
                .broadcast_to([128, 16, 128])
            )
            nc.vector.tensor_tensor(diag[:], id_bc, cw_bc, AluOp.mult)
            pb = bp.tile([128, 2, 4, 128], F32, name="pb", space="PSUM")
            for gg in range(4):
                g = gh * 4 + gg
                gt = gp.tile([128, 4, 256], BF16, name="gt")
                t = k * NG + g
                nc.gpsimd.indirect_dma_start(
                    out=gt[:].rearrange("p a b -> p (a b)"),
                    out_offset=None,
                    in_=xi,
                    in_offset=bass.IndirectOffsetOnAxis(
                        ap=ridx[:, t : t + 1], axis=0
                    ),
                )
                # blend + transpose: psum[ch,pos] += gt_a^T @ diag(w_a)
                for cc in range(2):
                    for a in range(4):
                        nc.tensor.matmul(
                            pb[:, cc, gg, :],
                            gt[:, a, cc * 128 : (cc + 1) * 128],
                            diag[:, 4 * gg + a, :],
                            start=(a == 0),
                            stop=(a == 3),
                        )
            nc.scalar.copy(rhsT[:, :, gh * 4 : gh * 4 + 4, :], pb[:])
        # conv: accumulate this tap's 2 cin-chunks into the persistent psum
        for h in range(2):
            for cc in range(2):
                j = 2 * k + cc
                for sh in range(2):
                    nc.tensor.matmul(
                        po[:, h, sh * 512 : (sh + 1) * 512],
                        wT_s[:, j, h * 128 : (h + 1) * 128],
                        rhsT[:, cc].rearrange("p g b -> p (g b)")[
                            :, sh * 512 : (sh + 1) * 512
                        ],
                        start=(j == 0),
                        stop=(j == 17),
                    )
    for h in range(2):
        for sh in range(2):
            osb = op.tile([128, 512], F32, name=f"osb{h}{sh}")
            nc.scalar.copy(osb[:], po[:, h, sh * 512 : (sh + 1) * 512])
            nc.sync.dma_start(out[:, h, sh * 512 : (sh + 1) * 512], osb[:])

    ctx.close()


# ---------------- host-side prep ----------------

def core_inputs(x, offset, weight):
    """Full inputs (np f32) -> list of 8 per-core input dicts."""
    bf = ml_dtypes.bfloat16
    x = np.asarray(x, np.float32)
    offset = np.asarray(offset, np.float32)
    weight = np.asarray(weight, np.float32)

    # y-pair-interleaved channels-last images, bf16: xi[r] = [x[r], x[r+64]]
    xis = []
    for n in range(N):
        xcl = np.ascontiguousarray(x[n].reshape(CIN, S).T)  # [4096, 256]
        xi = np.zeros((S, 2 * CIN), np.float32)
        xi[:, :CIN] = xcl
        xi[: S - W, CIN:] = xcl[W:]
        xis.append(xi.astype(bf))

    # lhsT [k*256+c, o]
    wk = weight.reshape(COUT, CIN, K)           # [o, c, k]
    wT = np.ascontiguousarray(wk.transpose(2, 1, 0).reshape(K * CIN, COUT)).astype(bf)

    ident = np.eye(128, dtype=bf)

    # sample coordinates: py/px [K, S] per batch
    off = offset.reshape(N, K, 2, S)
    ky, kx = np.meshgrid(np.arange(KH), np.arange(KW), indexing="ij")
    ky = ky.reshape(K, 1).astype(np.float32)
    kx = kx.reshape(K, 1).astype(np.float32)
    ho, wo = np.meshgrid(np.arange(H), np.arange(W), indexing="ij")
    base_y = ho.reshape(1, S).astype(np.float32) - 1.0 + ky   # [K, S]
    base_x = wo.reshape(1, S).astype(np.float32) - 1.0 + kx

    ins = []
    for core in range(8):
        n, qtr = core // 4, core % 4
        sl = slice(qtr * SLOC, (qtr + 1) * SLOC)
        py = base_y[:, sl] + off[n, :, 0, sl]   # [K, 1024]
        px = base_x[:, sl] + off[n, :, 1, sl]

        fy = np.floor(py)
        fx = np.floor(px)
        ly, lx = py - fy, px - fx
        hy, hx = 1.0 - ly, 1.0 - lx
        wy_c = np.clip(fy, 0.0, 62.0)           # window start rows
        wx_c = np.clip(fx, 0.0, 62.0)

        def sw(f, l, h, wc):
            """weights of window slots 0/1 along one axis, validity folded."""
            v0 = (f >= 0) & (f <= 63)
            v1 = (f + 1 >= 0) & (f + 1 <= 63)
            w0 = h * v0                          # corner f
            w1 = l * v1                          # corner f+1
            ws = []
            for s_ in (0, 1):
                c = wc + s_
                ws.append(w0 * (c == f) + w1 * (c == f + 1))
            return ws                            # [2][K, 1024]

        wys = sw(fy, ly, hy, wy_c)
        wxs = sw(fx, lx, hx, wx_c)
        rows = (wy_c * 64.0 + wx_c).astype(np.int32)        # [K, 1024]

        # device layouts: position p*8+g <-> (partition p, group g)
        def lay(a):                               # [K, 1024] -> [128, K, 8]
            return np.ascontiguousarray(
                a.reshape(K, 128, NG).transpose(1, 0, 2)
            )

        ridx = lay(rows).reshape(128, K * NG).astype(np.int32)  # col k*8+g
        cwa = np.stack(
            [wys[0] * wxs[0], wys[1] * wxs[0], wys[0] * wxs[1], wys[1] * wxs[1]],
            axis=0,
        )                                          # [4, K, 1024]
        # -> [128, K, 8*4+a]  (column within tap = g*4 + a)
        cwk = np.ascontiguousarray(
            cwa.reshape(4, K, 128, NG).transpose(2, 1, 3, 0).reshape(128, K, 4 * NG)
        ).astype(bf)

        ins.append({
            "xi": xis[n],
            "wT": wT,
            "ident": ident,
            "ridx": ridx,
            "cw": cwk,
        })
    return ins


def assemble(results):
    """list of 8 per-core {'out': [128,2,1024] f32} -> [2,256,64,64] f32."""
    out = np.zeros((N, COUT, S), np.float32)
    for core in range(8):
        n, qtr = core // 4, core % 4
        o = np.asarray(results[core]["out"])          # [oc, h, g*128+p]
        o = o.transpose(1, 0, 2).reshape(COUT, SLOC)  # [cout, g*128+p]
        # position within quarter = p*8+g
        o = o.reshape(COUT, NG, 128).transpose(0, 2, 1).reshape(COUT, SLOC)
        out[n, :, qtr * SLOC : (qtr + 1) * SLOC] = o
    return out.reshape(N, COUT, H, W)


def declare_io(nc):
    ins = {
        "xi": nc.dram_tensor("xi", [S, 2 * CIN], BF16, kind="ExternalInput").ap(),
        "wT": nc.dram_tensor("wT", [K * CIN, COUT], BF16, kind="ExternalInput").ap(),
        "ridx": nc.dram_tensor("ridx", [128, NT], I32, kind="ExternalInput").ap(),
        "cw": nc.dram_tensor("cw", [128, K, 4 * NG], BF16, kind="ExternalInput").ap(),
        "ident": nc.dram_tensor("ident", [128, 128], BF16, kind="ExternalInput").ap(),
    }
    outs = {
        "out": nc.dram_tensor("out", [128, 2, SLOC], F32, kind="ExternalOutput").ap(),
    }
    return outs, ins


def build_module():
    from concourse import bacc

    nc = bacc.Bacc(
        "TRN2",
        target_bir_lowering=False,
        debug=False,
        num_devices=8,
        dynamic_dma_scratch_size=65536,
    )
    outs, ins = declare_io(nc)
    with tile.TileContext(nc) as tc:
        build_core_kernel(nc, tc, outs, ins)
    nc.compile()
    return nc


_NC_CACHE = []


def kernel(x, offset, weight):
    """Full (unsharded) inputs -> full output, computed on 8 NeuronCores."""
    import time

    from concourse.bass_utils import run_bass_kernel_spmd

    if not _NC_CACHE:
        _NC_CACHE.append(build_module())
    nc = _NC_CACHE[0]
    core_ins = core_inputs(x, offset, weight)
    last = None
    for attempt in range(3):
        try:
            res = run_bass_kernel_spmd(nc, core_ins, core_ids=list(range(8)))
            return assemble(res.results)
        except Exception as e:  # transient device-session failures
            last = e
            time.sleep(2.0 * (attempt + 1))
    raise last


# revision 14
# speedup vs baseline: 1.2353x; 1.0031x over previous
"""Self-contained Trainium2 Bass kernel for deformable conv 2d.

kernel(x, offset, weight) -> out, matching the jax reference:
  x[2,256,64,64] f32, offset[2,18,64,64] f32, weight[256,256,3,3] f32
  -> out[2,256,64,64] f32 (KH=KW=3, stride=1, pad=1, dil=1, DG=1).

Runs SPMD on 8 NeuronCores, data-parallel: core = (batch, spatial quarter).

Device pipeline (per core, taps outer, 8 position-groups of 128 inner):
  - per tap k: DVE builds diag(w_corner) tiles for all 8 groups from
    host-computed bilinear weights (one broadcast-AP tensor_tensor).
  - 8 indirect gathers (one per group) fetch each sample's 2x2 bilinear
    window (4 corners x 256ch bf16) into [128 pos, 4*256].
  - PE does blend+transpose in one step: psum[ch,pos] += gt_a^T @ diag(w_a)
    accumulated over the 4 corners (regular matmul with diagonal rhs
    scales each transposed column by its sample weight).
  - Act copies blended psum -> SBUF bf16 rhsT; PE contracts the tap's
    (cin-chunk) pieces with the conv weights into the persistent psum out
    at 512-wide free dim.
Host precomputes gather indices and corner weights from the offsets.
"""

import sys

for _p in ("/opt/trn_rl_repo",):
    if _p not in sys.path:
        sys.path.insert(0, _p)


import numpy as np
import ml_dtypes

import concourse.bass as bass
import concourse.mybir as mybir
import concourse.tile as tile

F32 = mybir.dt.float32
BF16 = mybir.dt.bfloat16
I32 = mybir.dt.int32

N, CIN, H, W = 2, 256, 64, 64
COUT = 256
KH = KW = 3
K = KH * KW
S = H * W            # 4096 output positions per batch
SLOC = S // 4        # 1024 per core
NG = 8               # position groups per core (128 positions each)
NT = K * NG          # 72 (tap, group) gather slots

AluOp = mybir.AluOpType


def build_core_kernel(nc, tc, outs, ins):
    """Emit the per-core kernel. ins/outs are dicts of DRAM APs."""
    from contextlib import ExitStack

    xi = ins["xi"]          # [4096, 512] bf16 y-pair-interleaved image
    wT = ins["wT"]          # [2304, 256] bf16 lhsT
    ridx_d = ins["ridx"]    # [128, 72] i32 gather rows, col = k*8+g
    cw_d = ins["cw"]        # [128, 9, 32] bf16 corner weights (k; g*4+a)
    ident_d = ins["ident"]  # [128, 128] bf16 identity
    out = outs["out"]       # [128, 2, 1024] f32  (pos index = g*128+p)

    ctx = ExitStack()
    sp = ctx.enter_context(tc.tile_pool(name="static", bufs=1))
    gp = ctx.enter_context(tc.tile_pool(name="gather", bufs=12))
    dgp = ctx.enter_context(tc.tile_pool(name="diag", bufs=4))  # half-tap tiles
    rp = ctx.enter_context(tc.tile_pool(name="rhsT", bufs=2))
    bp = ctx.enter_context(tc.tile_pool(name="bpsum", bufs=2, space="PSUM"))
    cp = ctx.enter_context(tc.tile_pool(name="cpsum", bufs=1, space="PSUM"))
    op = ctx.enter_context(tc.tile_pool(name="osb", bufs=1))

    # ---- static loads (small control inputs first) ----
    ridx = sp.tile([128, NT], I32, name="ridx")
    nc.sync.dma_start(ridx[:], ridx_d)
    cw = sp.tile([128, K, 4 * NG], BF16, name="cw")
    nc.sync.dma_start(cw[:], cw_d)
    ident = sp.tile([128, 128], BF16, name="ident")
    nc.sync.dma_start(ident[:], ident_d)
    wT_s = sp.tile([128, 18, 256], BF16, name="wT_s")
    nc.sync.dma_start(wT_s[:], wT.rearrange("(j p) o -> p j o", p=128))

    id_bc = ident[:].unsqueeze(1).broadcast_to([128, 16, 128])

    po = cp.tile([128, 2, SLOC], F32, name="po", space="PSUM")

    def emit_conv(k, rhsT):
        """Contract tap k's 2 cin-chunks into the persistent psum out."""
        for h in range(2):
            for cc in range(2):
                j = 2 * k + cc
                for sh in range(2):
                    nc.tensor.matmul(
                        po[:, h, sh * 512 : (sh + 1) * 512],
                        wT_s[:, j, h * 128 : (h + 1) * 128],
                        rhsT[:, cc].rearrange("p g b -> p (g b)")[
                            :, sh * 512 : (sh + 1) * 512
                        ],
                        start=(j == 0),
                        stop=(j == 17),
                    )

    prev = None
    for k in range(K):
        rhsT = rp.tile([128, 2, NG, 128], BF16, name="rhsT")
        for gh in range(2):            # groups in halves of 4
            # diag tiles for this half: diag[p, (g,a), f] = I[p,f]*cw[p,k,(g,a)]
            diag = dgp.tile([128, 16, 128], BF16, name="diag")
            cw_bc = (
                cw[:, k, gh * 16 : gh * 16 + 16]
                .unsqueeze(-1)
                .broadcast_to([128, 16, 128])
            )
            nc.vector.tensor_tensor(diag[:], id_bc, cw_bc, AluOp.mult)
            pb = bp.tile([128, 2, 4, 128], F32, name="pb", space="PSUM")
            for gg in range(4):
                g = gh * 4 + gg
                gt = gp.tile([128, 4, 256], BF16, name="gt")
                t = k * NG + g
                nc.gpsimd.indirect_dma_start(
                    out=gt[:].rearrange("p a b -> p (a b)"),
                    out_offset=None,
                    in_=xi,
                    in_offset=bass.IndirectOffsetOnAxis(
                        ap=ridx[:, t : t + 1], axis=0
                    ),
                )
                # blend + transpose: psum[ch,pos] += gt_a^T @ diag(w_a)
                for cc in range(2):
                    for a in range(4):
                        nc.tensor.matmul(
                            pb[:, cc, gg, :],
                            gt[:, a, cc * 128 : (cc + 1) * 128],
                            diag[:, 4 * gg + a, :],
                            start=(a == 0),
                            stop=(a == 3),
                        )
            nc.scalar.copy(rhsT[:, :, gh * 4 : gh * 4 + 4, :], pb[:])
            # software pipelining: run the previous tap's conv between
            # this tap's blend halves so PE never waits on the Act copy
            if gh == 0 and prev is not None:
                emit_conv(k - 1, prev)
        prev = rhsT
    emit_conv(K - 1, prev)
    for h in range(2):
        for sh in range(2):
            osb = op.tile([128, 512], F32, name=f"osb{h}{sh}")
            nc.scalar.copy(osb[:], po[:, h, sh * 512 : (sh + 1) * 512])
            nc.sync.dma_start(out[:, h, sh * 512 : (sh + 1) * 512], osb[:])

    ctx.close()


# ---------------- host-side prep ----------------

def core_inputs(x, offset, weight):
    """Full inputs (np f32) -> list of 8 per-core input dicts."""
    bf = ml_dtypes.bfloat16
    x = np.asarray(x, np.float32)
    offset = np.asarray(offset, np.float32)
    weight = np.asarray(weight, np.float32)

    # y-pair-interleaved channels-last images, bf16: xi[r] = [x[r], x[r+64]]
    xis = []
    for n in range(N):
        xcl = np.ascontiguousarray(x[n].reshape(CIN, S).T)  # [4096, 256]
        xi = np.zeros((S, 2 * CIN), np.float32)
        xi[:, :CIN] = xcl
        xi[: S - W, CIN:] = xcl[W:]
        xis.append(xi.astype(bf))

    # lhsT [k*256+c, o]
    wk = weight.reshape(COUT, CIN, K)           # [o, c, k]
    wT = np.ascontiguousarray(wk.transpose(2, 1, 0).reshape(K * CIN, COUT)).astype(bf)

    ident = np.eye(128, dtype=bf)

    # sample coordinates: py/px [K, S] per batch
    off = offset.reshape(N, K, 2, S)
    ky, kx = np.meshgrid(np.arange(KH), np.arange(KW), indexing="ij")
    ky = ky.reshape(K, 1).astype(np.float32)
    kx = kx.reshape(K, 1).astype(np.float32)
    ho, wo = np.meshgrid(np.arange(H), np.arange(W), indexing="ij")
    base_y = ho.reshape(1, S).astype(np.float32) - 1.0 + ky   # [K, S]
    base_x = wo.reshape(1, S).astype(np.float32) - 1.0 + kx

    ins = []
    for core in range(8):
        n, qtr = core // 4, core % 4
        sl = slice(qtr * SLOC, (qtr + 1) * SLOC)
        py = base_y[:, sl] + off[n, :, 0, sl]   # [K, 1024]
        px = base_x[:, sl] + off[n, :, 1, sl]

        fy = np.floor(py)
        fx = np.floor(px)
        ly, lx = py - fy, px - fx
        hy, hx = 1.0 - ly, 1.0 - lx
        wy_c = np.clip(fy, 0.0, 62.0)           # window start rows
        wx_c = np.clip(fx, 0.0, 62.0)

        def sw(f, l, h, wc):
            """weights of window slots 0/1 along one axis, validity folded."""
            v0 = (f >= 0) & (f <= 63)
            v1 = (f + 1 >= 0) & (f + 1 <= 63)
            w0 = h * v0                          # corner f
            w1 = l * v1                          # corner f+1
            ws = []
            for s_ in (0, 1):
                c = wc + s_
                ws.append(w0 * (c == f) + w1 * (c == f + 1))
            return ws                            # [2][K, 1024]

        wys = sw(fy, ly, hy, wy_c)
        wxs = sw(fx, lx, hx, wx_c)
        rows = (wy_c * 64.0 + wx_c).astype(np.int32)        # [K, 1024]

        # device layouts: position p*8+g <-> (partition p, group g)
        def lay(a):                               # [K, 1024] -> [128, K, 8]
            return np.ascontiguousarray(
                a.reshape(K, 128, NG).transpose(1, 0, 2)
            )

        ridx = lay(rows).reshape(128, K * NG).astype(np.int32)  # col k*8+g
        cwa = np.stack(
            [wys[0] * wxs[0], wys[1] * wxs[0], wys[0] * wxs[1], wys[1] * wxs[1]],
            axis=0,
        )                                          # [4, K, 1024]
        # -> [128, K, 8*4+a]  (column within tap = g*4 + a)
        cwk = np.ascontiguousarray(
            cwa.reshape(4, K, 128, NG).transpose(2, 1, 3, 0).reshape(128, K, 4 * NG)
        ).astype(bf)

        ins.append({
            "xi": xis[n],
            "wT": wT,
            "ident": ident,
            "ridx": ridx,
            "cw": cwk,
        })
    return ins


def assemble(results):
    """list of 8 per-core {'out': [128,2,1024] f32} -> [2,256,64,64] f32."""
    out = np.zeros((N, COUT, S), np.float32)
    for core in range(8):
        n, qtr = core // 4, core % 4
        o = np.asarray(results[core]["out"])          # [oc, h, g*128+p]
        o = o.transpose(1, 0, 2).reshape(COUT, SLOC)  # [cout, g*128+p]
        # position within quarter = p*8+g
        o = o.reshape(COUT, NG, 128).transpose(0, 2, 1).reshape(COUT, SLOC)
        out[n, :, qtr * SLOC : (qtr + 1) * SLOC] = o
    return out.reshape(N, COUT, H, W)


def declare_io(nc):
    ins = {
        "xi": nc.dram_tensor("xi", [S, 2 * CIN], BF16, kind="ExternalInput").ap(),
        "wT": nc.dram_tensor("wT", [K * CIN, COUT], BF16, kind="ExternalInput").ap(),
        "ridx": nc.dram_tensor("ridx", [128, NT], I32, kind="ExternalInput").ap(),
        "cw": nc.dram_tensor("cw", [128, K, 4 * NG], BF16, kind="ExternalInput").ap(),
        "ident": nc.dram_tensor("ident", [128, 128], BF16, kind="ExternalInput").ap(),
    }
    outs = {
        "out": nc.dram_tensor("out", [128, 2, SLOC], F32, kind="ExternalOutput").ap(),
    }
    return outs, ins


def build_module():
    from concourse import bacc

    nc = bacc.Bacc(
        "TRN2",
        target_bir_lowering=False,
        debug=False,
        num_devices=8,
        dynamic_dma_scratch_size=65536,
    )
    outs, ins = declare_io(nc)
    with tile.TileContext(nc) as tc:
        build_core_kernel(nc, tc, outs, ins)
    nc.compile()
    return nc


_NC_CACHE = []


def kernel(x, offset, weight):
    """Full (unsharded) inputs -> full output, computed on 8 NeuronCores."""
    import time

    from concourse.bass_utils import run_bass_kernel_spmd

    if not _NC_CACHE:
        _NC_CACHE.append(build_module())
    nc = _NC_CACHE[0]
    core_ins = core_inputs(x, offset, weight)
    last = None
    for attempt in range(3):
        try:
            res = run_bass_kernel_spmd(nc, core_ins, core_ids=list(range(8)))
            return assemble(res.results)
        except Exception as e:  # transient device-session failures
            last = e
            time.sleep(2.0 * (attempt + 1))
    raise last
